# revision 1
# baseline (speedup 1.0000x reference)
"""HetConv (3x3 block-diagonal-by-residue + 1x1 elsewhere) on 8 trn2 cores.

Strategy: data-parallel over batch (4 images/core, weights replicated).
Per core: implicit-GEMM conv over a 66x66 zero-padded SBUF image with
channels permuted by residue mod 4 (done via strided DMA access patterns,
never materialized on host). Effective weight packs into 20 [128x128]
float32r matmul slots per spatial tile instead of 36 dense ones:
  - slots 9c+ti, c in {0,1}, ti in 0..8: tap (ky,kx)=divmod(ti,3), block-diag
    Wk for groups (2c, 2c+1); center tap also carries same-chunk W1 in its
    off-diagonal blocks.
  - slot 18/19: cross-chunk center-tap W1 (chunk0->oc chunk1 and reverse).
"""
import sys

sys.path.insert(0, "/opt/trn_rl_repo")

import numpy as np
import concourse.bacc as bacc
import concourse.mybir as mybir
from concourse import tile
from concourse.bass_utils import run_bass_kernel_spmd

N_CORES = 8
B, C, H, W = 32, 256, 64, 64
BP = B // N_CORES          # images per core
HP, WP = H + 2, W + 2      # padded image
NTILES = 8                 # output row-bands per image
RPT = H // NTILES          # rows per band
NFREE = RPT * W            # matmul moving free size (512)
NSLOTS = 20

_PROG = None


def _build(reps=1, packed=False):
    nc = bacc.Bacc("TRN2", target_bir_lowering=False, debug=False,
                   num_devices=N_CORES)
    f32 = mybir.dt.float32
    f32r = mybir.dt.float32r

    # x arrives host-padded to [BP, C, 66, 66] (zero border) so the whole
    # padded image DMAs as one contiguous run per partition.
    x = nc.dram_tensor("x", [BP, C, HP * WP], f32r, kind="ExternalInput").ap()
    w = nc.dram_tensor("w", [128, NSLOTS * 128], f32r, kind="ExternalInput").ap()
    out = nc.dram_tensor("out", [BP, C, H, W], f32, kind="ExternalOutput").ap()

    # channel c = 4k + g  ->  [b, g, k, ...]
    x_r = x.rearrange("b (k four) s -> b four k s", four=4)
    out_r = out.rearrange("b (k four) h w -> b four k h w", four=4)

    with tile.TileContext(nc) as tc:
        with (
            tc.tile_pool(name="wpool", bufs=1) as wpool,
            tc.tile_pool(name="xpool", bufs=2) as xpool,
            tc.tile_pool(name="opool", bufs=3) as opool,
            tc.tile_pool(name="pspool", bufs=3, space="PSUM") as pspool,
        ):
            wt = wpool.tile([128, NSLOTS * 128], f32r)
            nc.sync.dma_start(out=wt[:, :], in_=w[:, :])

            def wslot(s):
                return wt[:, s * 128:(s + 1) * 128]

            for img in [i % BP for i in range(BP * reps)]:
                xvs = []
                for cchunk in (0, 1):
                    xp = xpool.tile([128, HP * WP], f32r,
                                    tag=f"xp{cchunk}")
                    # partitions 0-63 <- residue 2c, 64-127 <- 2c+1; one
                    # fully-contiguous DMA per (img, chunk)
                    nc.gpsimd.dma_start(
                        out=xp[:, :],
                        in_=x_r[img, 2 * cchunk:2 * cchunk + 2],
                    )
                    xvs.append(xp[:, :].rearrange("p (h w) -> p h w", w=WP))

                for nt in range(NTILES):
                    def rhs(cchunk, ky, kx):
                        return xvs[cchunk][:, nt * RPT + ky: nt * RPT + ky + RPT,
                                           kx:kx + W]

                    for oc_chunk in (0, 1):
                        ps = pspool.tile([128, NFREE], f32, tag=f"ps{oc_chunk}")
                        if not packed:
                            for ti in range(9):
                                ky, kx = divmod(ti, 3)
                                nc.tensor.matmul(
                                    ps[:, :], wslot(9 * oc_chunk + ti),
                                    rhs(oc_chunk, ky, kx),
                                    start=(ti == 0), stop=False,
                                )
                        else:
                            # center tap first: full 128x128 (Wk diag + W1
                            # off-diag), start=True sets has_written everywhere
                            nc.tensor.matmul(
                                ps[:, :], wslot(9 * oc_chunk + 4),
                                rhs(oc_chunk, 1, 1), start=True, stop=False,
                            )
                            # non-center taps as row-strip pairs: each
                            # block-diag slot splits into two K=64, M=128
                            # matmuls on disjoint row strips (the slot's row
                            # halves are [W_geven | 0] and [0 | W_godd]).
                            # Adjacent row strips carry different taps, so
                            # the PE can overlap them (row tiling).
                            for t in (0, 1, 2, 3, 5, 6, 7, 8):
                                ky, kx = divmod(t, 3)
                                s = 9 * oc_chunk + t
                                r = rhs(oc_chunk, ky, kx)
                                nc.tensor.matmul(
                                    ps[:, :],
                                    wt[0:64, s * 128:(s + 1) * 128],
                                    r[0:64], start=False, stop=False,
                                    tile_position=(0, 0),
                                    skip_group_check=True,
                                )
                                nc.tensor.matmul(
                                    ps[:, :],
                                    wt[64:128, s * 128:(s + 1) * 128],
                                    r[64:128], start=False, stop=False,
                                    tile_position=(64, 0),
                                    skip_group_check=True,
                                )
                        # cross-chunk center-tap W1: slot 18 is ic-chunk0 ->
                        # oc-chunk1, slot 19 the reverse
                        nc.tensor.matmul(
                            ps[:, :], wslot(19 - oc_chunk),
                            rhs(1 - oc_chunk, 1, 1),
                            start=False, stop=True,
                        )
                        ot = opool.tile([128, NFREE], f32, tag=f"ot{oc_chunk}")
                        nc.vector.tensor_copy(ot[:, :], ps[:, :])
                        # one DMA per residue half, on separate queues (SP /
                        # ACT) so the two output streams run in parallel
                        engs = {(0, 0): nc.sync, (0, 1): nc.sync,
                                (1, 0): nc.scalar, (1, 1): nc.scalar}
                        for half in (0, 1):
                            g = 2 * oc_chunk + half
                            engs[(oc_chunk, half)].dma_start(
                                out=out_r[img, g, :, nt * RPT:(nt + 1) * RPT, :],
                                in_=ot[64 * half:64 * half + 64, :],
                            )

    nc.compile()
    return nc


def _get_prog():
    global _PROG
    if _PROG is None:
        _PROG = _build()
    return _PROG


def _prep_weights(Wk, W1):
    idx = [np.arange(g, 256, 4) for g in range(4)]
    wslabs = np.zeros((NSLOTS, 128, 128), np.float32)
    for c in (0, 1):
        gs = (2 * c, 2 * c + 1)
        for ti in range(9):
            ky, kx = divmod(ti, 3)
            s = 9 * c + ti
            for a in (0, 1):        # ic block position
                for b in (0, 1):    # oc block position
                    ga, gb = gs[a], gs[b]
                    if a == b:
                        blk = Wk[np.ix_(idx[gb], idx[ga])][:, :, ky, kx].T
                    elif ti == 4:
                        blk = W1[np.ix_(idx[gb], idx[ga])].T
                    else:
                        continue
                    wslabs[s, 64 * a:64 * a + 64, 64 * b:64 * b + 64] = blk
    for s, (ic_gs, oc_gs) in ((18, ((0, 1), (2, 3))), (19, ((2, 3), (0, 1)))):
        for a, ga in enumerate(ic_gs):
            for b, gb in enumerate(oc_gs):
                wslabs[s, 64 * a:64 * a + 64, 64 * b:64 * b + 64] = \
                    W1[np.ix_(idx[gb], idx[ga])].T
    # SBUF layout [K partition, slot*128 + m]
    return np.ascontiguousarray(
        wslabs.transpose(1, 0, 2).reshape(128, NSLOTS * 128))


def _make_in_maps(x, Wk, W1):
    w_host = _prep_weights(np.asarray(Wk, np.float32), np.asarray(W1, np.float32))
    xs = np.asarray(x, np.float32)
    xpad = np.zeros((B, C, HP, WP), np.float32)
    xpad[:, :, 1:H + 1, 1:W + 1] = xs
    xpad = xpad.reshape(B, C, HP * WP)
    return [
        {"x": np.ascontiguousarray(xpad[i * BP:(i + 1) * BP]), "w": w_host}
        for i in range(N_CORES)
    ]


def _run(x, Wk, W1, **spmd_kwargs):
    nc = _get_prog()
    in_maps = _make_in_maps(x, Wk, W1)
    res = run_bass_kernel_spmd(nc, in_maps, list(range(N_CORES)), **spmd_kwargs)
    outs = np.concatenate(
        [res.results[i]["out"] for i in range(N_CORES)], axis=0)
    return outs, res


def kernel(x, Wk, W1):
    return _run(x, Wk, W1)[0]

